# revision 37
# baseline (speedup 1.0000x reference)
"""Causal self-attention (B=2, T=2048, D=1024, H=16, HD=64) on 8 trn2 NeuronCores.

Sharding: core = b*4 + g  (b = batch 0..1, g = head-group 0..3, 4 heads each).
Megatron-style: column-split w_qkv per head group, row-split w_o; the w_o
all-reduce over each batch's 4 cores is done host-side (partial sums).

v2 device program (vs baseline):
  - Heads processed as 2 pairs; pair p stacks head 2p at partitions 0:64 and
    head 2p+1 at 64:128 (qk2 layout). Projections evict PSUM->SBUF directly
    on DVE (no SBUF->SBUF re-partition DMAs, no zero padding).
  - Scores: per tk-block, two concurrent row-tiled matmuls (K=64 contraction,
    PE tile positions (0,0) and (64,0)) -> both heads' scoresT [tk,tq] in one
    512-col stream.
  - Causality: diagonal 128-blocks only compute/exp/accumulate columns >=
    s_min; the partial triangle inside the leading 128 cols is zeroed post-exp
    with gpsimd affine_select (j - p >= 0), so no mask tiles and no PE mask
    matmuls.
  - Softmax denominator from the ones-column trick (v tile col 64); 1/denom
    via ACT Ln then Exp(scale=-1) (same table set as the softmax Exp, so no
    table reloads), broadcast with gpsimd, multiply on DVE.
  - Software pipelining: proj(c+1) and outproj(c-1) matmul groups are
    interleaved into chunk c's ACT-bound attention stream so the PE never
    drains (HAM stays warm). PSUM: shared "ps" ring (2 bufs x 2 banks) +
    "psy" ring (4 bufs x 1 bank) = exactly 8 banks.
"""

import os
import numpy as np

B, T, D = 2, 2048, 1024
H, HD = 16, 64
LH = 4            # local heads per core
KO = 8            # contraction tiles of 128 over D
DQK = 512         # q+k columns per core (4 heads * 64 * 2)
DVE_ = 260        # v columns per core incl. ones cols (4 * 65)
NTQ, TQC = 4, 512
NTKB, TKB = 16, 128

_PROG = {}
LAST_RESULT = None


def _build_program(debug_dumps=False):
    import concourse.bass as bass
    from concourse import bacc
    import concourse.tile as tile
    import concourse.mybir as mybir

    f32 = mybir.dt.float32
    f32r = mybir.dt.float32r
    bf16 = mybir.dt.bfloat16
    AF = mybir.ActivationFunctionType
    ALU = mybir.AluOpType
    ts = bass.ts

    nc = bacc.Bacc(None, target_bir_lowering=False, debug=True)
    xT_d = nc.dram_tensor("xT", [128, KO, T], bf16, kind="ExternalInput")
    wqk_d = nc.dram_tensor("w_qk", [128, KO, DQK], bf16, kind="ExternalInput")
    bqk_d = nc.dram_tensor("b_qk", [128, 4], f32, kind="ExternalInput")
    wv_d = nc.dram_tensor("w_v", [128, KO, DVE_], bf16, kind="ExternalInput")
    bv_d = nc.dram_tensor("b_v", [128, DVE_], f32, kind="ExternalInput")
    wo_d = nc.dram_tensor("w_o", [128, 2, D], bf16, kind="ExternalInput")
    out_d = nc.dram_tensor("out_part", [T, D], f32, kind="ExternalOutput")
    dbg = {}
    if debug_dumps:
        dbg["qk2"] = nc.dram_tensor("dbg_qk2", [128, 2, 2, T], mybir.dt.bfloat16, kind="ExternalOutput")
        dbg["v"] = nc.dram_tensor("dbg_v", [128, NTKB, LH, 65], mybir.dt.bfloat16, kind="ExternalOutput")
        dbg["yT"] = nc.dram_tensor("dbg_yT", [128, 2, T], mybir.dt.bfloat16, kind="ExternalOutput")

    with tile.TileContext(nc) as tc:
        with (
            tc.tile_pool(name="big", bufs=1) as big,
            tc.tile_pool(name="xtp", bufs=2) as xtp,
            tc.tile_pool(name="expp", bufs=4) as expp,
            tc.tile_pool(name="ev", bufs=2) as ev,
            tc.tile_pool(name="outp", bufs=3) as outp,
            tc.tile_pool(name="ps", bufs=2, space="PSUM") as ps,
            tc.tile_pool(name="psy", bufs=4, space="PSUM") as psyp,
        ):
            wqk = big.tile([128, KO, DQK], bf16, name="wqk_sb")
            wv = big.tile([128, KO, DVE_], bf16, name="wv_sb")
            wo = big.tile([128, 2, D], bf16, name="wo_sb")
            bqk = big.tile([128, 4], f32, name="bqk_sb")
            bv = big.tile([128, LH, 65], f32, name="bv_sb")
            # qk2[0:64, p, qk, t]   = head 2p   (qk=0 -> q, qk=1 -> k)
            # qk2[64:128, p, qk, t] = head 2p+1
            qk2 = big.tile([128, 2, 2, T], bf16, name="qk2_sb")
            vsb = big.tile([128, NTKB, LH, 65], f32r, name="v_sb")
            # yT[0:64, p, t] = y of head 2p; yT[64:128, p, t] = head 2p+1
            yT = big.tile([128, 2, T], bf16, name="yT_sb")

            # startup DMAs in first-need order.  Two HWDGE rings: bulk loads
            # (x chunks, weights) go on the scalar ring so the sync ring
            # stays free for small latency-critical DMAs (normalize chain,
            # output stores never queue behind a 2MB prefetch).
            # per-ko slices so the first projection matmuls start as soon as
            # their contraction slice lands instead of after the full 4MB
            xts = {}
            xts[0] = xtp.tile([128, KO, 512], bf16, name="xTc_0", tag="xTc")
            for ko in range(KO):
                nc.sync.dma_start(wqk[:, ko, :], wqk_d[:, ko, :])
                nc.scalar.dma_start(xts[0][:, ko, :], xT_d[:, ko, 0:512])
            nc.sync.dma_start(bqk[:], bqk_d[:])
            nc.scalar.dma_start(wv[:], wv_d[:])
            nc.sync.dma_start(bv[:], bv_d[:].rearrange("p (h e) -> p h e", h=LH))
            nc.scalar.dma_start(wo[:], wo_d[:])

            # warm the gpsimd extended-instruction library (partition
            # broadcast + SWDGE) so the one-time ~6us IRAM load happens
            # during the prologue DMAs, not inside the first normalize.
            wrm = ev.tile([64, 4], f32, name="wrm", tag="wrm")
            nc.gpsimd.partition_broadcast(wrm[:], bqk[0:1, :])
            wrm2 = ev.tile([64, 4], f32, name="wrm2", tag="wrm2")
            nc.gpsimd.dma_start(wrm2[:], wrm[:])

            def prefetch_x(c):
                xts[c] = xtp.tile([128, KO, 512], bf16, name=f"xTc_{c}", tag="xTc")
                nc.scalar.dma_start(xts[c][:], xT_d[:, :, ts(c, 512)])

            def proj_items(c):
                """8 closures: 4 q/k subtile groups + 4 v tk-block groups."""
                items = []
                for s in range(4):
                    def qk_item(s=s, c=c):
                        pst = ps.tile([128, 512], f32, name=f"pp_{s}_{c}", tag="ps")
                        for ko in range(KO):
                            nc.tensor.matmul(
                                pst[:],
                                wqk[:, ko, ts(s, 128)],
                                xts[c][:, ko, :],
                                start=(ko == 0),
                                stop=(ko == KO - 1),
                            )
                        p, qki = s % 2, s // 2
                        nc.vector.tensor_scalar_add(
                            qk2[:, p, qki, ts(c, 512)], pst[:], bqk[:, s : s + 1]
                        )
                    items.append(qk_item)
                for tbl in range(4):
                    def v_item(tbl=tbl, c=c):
                        tb = 4 * c + tbl
                        pst = ps.tile([128, DVE_], f32, name=f"pv_{tb}", tag="ps")
                        for ko in range(KO):
                            nc.tensor.matmul(
                                pst[:],
                                xts[c][:, ko, ts(tbl, 128)],
                                wv[:, ko, :],
                                start=(ko == 0),
                                stop=(ko == KO - 1),
                            )
                        nc.vector.tensor_add(
                            vsb[:, tb, :, :],
                            pst[:].rearrange("p (h e) -> p h e", h=LH),
                            bv[:],
                        )
                    items.append(v_item)
                return items

            def outproj_items(c, use_act=False):
                items = []
                for mb_ in range(4):
                    for n in range(2):
                        def o_item(mb_=mb_, n=n, c=c, use_act=use_act):
                            m = 4 * c + mb_
                            if use_act:
                                # epilogue: attention is done, the psy pool's
                                # 4 banks are free -> deeper outproj pipeline
                                pst = psyp.tile(
                                    [128, 512], f32, name=f"po_{m}_{n}", tag="psy"
                                )
                            else:
                                pst = ps.tile(
                                    [128, 512], f32, name=f"po_{m}_{n}", tag="ps"
                                )
                            for kt in range(2):
                                nc.tensor.matmul(
                                    pst[:],
                                    yT[:, kt, ts(m, 128)],
                                    wo[:, kt, ts(n, 512)],
                                    start=(kt == 0),
                                    stop=(kt == 1),
                                )
                            ot = outp.tile(
                                [128, 512], f32, name=f"ot_{m}_{n}", tag="ot"
                            )
                            if use_act and n == 1:
                                nc.scalar.copy(ot[:], pst[:])
                            else:
                                nc.vector.tensor_copy(ot[:], pst[:])
                            nc.sync.dma_start(out_d[ts(m, 128), ts(n, 512)], ot[:])
                        items.append(o_item)
                return items

            def attention_steps(c):
                """Closures: per pair, nb block-steps then a normalize step."""
                nb = 4 * (c + 1)
                steps = []
                psys = {}

                def make_block(p, tb):
                    def block(p=p, tb=tb):
                        if tb == 0:
                            for e in range(2):
                                h = 2 * p + e
                                psys[h] = psyp.tile(
                                    [128, 512], f32, name=f"psy_{h}_{c}", tag="psy"
                                )
                        smin_t = max(0, 128 * tb - 512 * c)  # true causal start
                        smin = min(smin_t, 256)  # f32r matmuls need N >= 256
                        pss = ps.tile(
                            [128, 2, 512], f32, name=f"pss_{p}_{tb}_{c}", tag="ps"
                        )
                        for e in range(2):
                            lo, hi = (0, 64) if e == 0 else (64, 128)
                            nc.tensor.matmul(
                                pss[:, e, smin:512],
                                qk2[lo:hi, p, 1, ts(tb, 128)],
                                qk2[lo:hi, p, 0, 512 * c + smin : 512 * (c + 1)],
                                start=True,
                                stop=True,
                            )
                        et = expp.tile(
                            [128, 2, 512], f32r, name=f"et_{p}_{tb}_{c}", tag="et"
                        )
                        nc.scalar.activation(
                            et[:, :, smin:512],
                            pss[:, :, smin:512],
                            AF.Exp,
                            scale=1.0 / 32.0,
                        )
                        if 128 * tb >= 512 * c:  # diagonal block: zero the triangle
                            w = 128 + (smin_t - smin)
                            nc.gpsimd.affine_select(
                                et[:, :, smin : smin + w],
                                et[:, :, smin : smin + w],
                                pattern=[[0, 2], [1, w]],
                                compare_op=ALU.is_ge,
                                fill=0.0,
                                base=smin - smin_t,
                                channel_multiplier=-1,
                            )
                        for e in range(2):
                            h = 2 * p + e
                            nc.tensor.matmul(
                                psys[h][0:65, smin:512],
                                vsb[:, tb, h, :],
                                et[:, e, smin:512],
                                start=(tb == 0),
                                stop=(tb == nb - 1),
                            )
                    return block

                def make_norm(p):
                    def norm(p=p):
                        # gather the two denominator rows (partition 64 of each
                        # psy bank) into partitions 0:2 (one copy on DVE, one
                        # on ACT so they run concurrently)
                        dcp = ev.tile([128, 2, 512], f32, name=f"dcp_{p}_{c}", tag="dcp")
                        nc.vector.tensor_copy(dcp[64:65, 0, :], psys[2 * p][64:65, :])
                        nc.scalar.copy(dcp[64:65, 1, :], psys[2 * p + 1][64:65, :])
                        # reshape [1,2,512] -> [64,16] so the free-size-bound
                        # DVE reciprocal runs 32x wider; then reshape back.
                        den = ev.tile([64, 16], f32, name=f"den_{p}_{c}", tag="den")
                        nc.sync.dma_start(den[:, :], dcp[64:65, :, :])
                        rcw = ev.tile([64, 16], f32, name=f"rcw_{p}_{c}", tag="rcw")
                        nc.vector.reciprocal(rcw[:], den[:])
                        rcp = ev.tile([1, 2, 512], f32, name=f"rcp_{p}_{c}", tag="rcp")
                        nc.sync.dma_start(rcp[:, :, :], rcw[:, :])
                        # one broadcast covers both heads
                        rb = ev.tile([64, 2, 512], f32, name=f"rb_{p}_{c}", tag="rb")
                        nc.gpsimd.partition_broadcast(rb[:], rcp[0:1, :, :])
                        for e in range(2):
                            h = 2 * p + e
                            if e == 0:
                                nc.vector.tensor_mul(
                                    yT[0:64, p, ts(c, 512)],
                                    psys[h][0:64, :],
                                    rb[:, 0, :],
                                )
                            else:
                                tmp = ev.tile(
                                    [64, 512], bf16, name=f"tmp_{h}_{c}", tag="tmpy"
                                )
                                nc.vector.tensor_mul(
                                    tmp[:], psys[h][0:64, :], rb[:, 1, :]
                                )
                                nc.gpsimd.dma_start(
                                    yT[64:128, p, ts(c, 512)], tmp[:]
                                )
                    return norm

                for p in range(2):
                    for tb in range(nb):
                        steps.append(make_block(p, tb))
                    if p == 0:
                        steps.append(make_norm(0))
                # final normalize is returned separately: it must be emitted
                # AFTER all interleaved extras, else its long DMA-latency
                # chain head-of-line-blocks the DVE/sync queues ahead of the
                # extras' evictions that the next chunk depends on.
                return steps, make_norm(1)

            # prologue: projections for chunk 0
            prefetch_x(1)
            for it in proj_items(0):
                it()

            # outproj(c) becomes available during chunk c+1; drain the pending
            # queue preferentially in the LATER (longer, ACT-paced) chunks
            # where the PE has slack under the exp stream.  quota[c] = number
            # of outproj items to emit during chunk c.
            pending_out = []
            quota = {0: 0, 1: 4, 2: 8, 3: 12}
            for c in range(NTQ):
                if c + 2 < NTQ:
                    prefetch_x(c + 2)
                steps, final_norm = attention_steps(c)
                if c - 1 >= 0:
                    pending_out += outproj_items(c - 1)
                extras = []
                if c + 1 < NTQ:
                    extras += proj_items(c + 1)
                nq = min(quota[c], len(pending_out))
                extras += pending_out[:nq]
                pending_out = pending_out[nq:]
                ne, ns = len(extras), len(steps)
                j = 0
                for i, st in enumerate(steps):
                    st()
                    while j < ne and j * ns < (i + 1) * ne:
                        extras[j]()
                        j += 1
                final_norm()

            # epilogue: remaining deferred outproj + the last chunk's
            pending_out += outproj_items(NTQ - 1, use_act=True)
            for it in pending_out:
                it()

            if debug_dumps:
                nc.sync.dma_start(dbg["qk2"][:], qk2[:])
                nc.sync.dma_start(dbg["v"][:], vsb[:])
                nc.sync.dma_start(dbg["yT"][:], yT[:])

    nc.finalize()
    return nc


def kernel(x, w_qkv, b_qkv, w_o, b_o):
    global LAST_RESULT
    from concourse.bass_utils import run_bass_kernel_spmd

    x = np.asarray(x, dtype=np.float32)
    w_qkv = np.asarray(w_qkv, dtype=np.float32)
    b_qkv = np.asarray(b_qkv, dtype=np.float32)
    w_o = np.asarray(w_o, dtype=np.float32)
    b_o = np.asarray(b_o, dtype=np.float32)

    if "nc" not in _PROG:
        _PROG["nc"] = _build_program()
    nc = _PROG["nc"]

    import ml_dtypes

    # host-side shard prep
    xT = []
    for b in range(B):
        t = np.ascontiguousarray(x[b].T)  # [D, T]
        xT.append(
            np.ascontiguousarray(
                t.reshape(KO, 128, T).swapaxes(0, 1).astype(ml_dtypes.bfloat16)
            )
        )

    in_maps = []
    for core in range(8):
        b, g = divmod(core, 4)
        # w_qk subtile order: s=0: q heads(0,1); s=1: q heads(2,3);
        #                     s=2: k heads(0,1); s=3: k heads(2,3)
        cols = []
        bvals = []
        for qki in range(2):  # 0=q, 1=k
            for p in range(2):
                for e in range(2):
                    h = 2 * p + e
                    sl = slice(
                        qki * D + g * 256 + h * 64, qki * D + g * 256 + (h + 1) * 64
                    )
                    cols.append(w_qkv[:, sl])
                    bvals.append(b_qkv[sl])
        w_qk = np.concatenate(cols, axis=1)  # [D, 512]
        w_qk = np.ascontiguousarray(
            w_qk.reshape(KO, 128, DQK).swapaxes(0, 1).astype(ml_dtypes.bfloat16)
        )
        b_qk = np.concatenate(bvals)  # [512]
        b_qk = np.ascontiguousarray(b_qk.reshape(4, 128).T)  # [128, 4]

        w_v = np.zeros((D, DVE_), dtype=np.float32)
        b_v = np.zeros((DVE_,), dtype=np.float32)
        for h in range(LH):
            vcols = slice(2 * D + g * 256 + h * 64, 2 * D + g * 256 + (h + 1) * 64)
            w_v[:, h * 65 : h * 65 + 64] = w_qkv[:, vcols]
            b_v[h * 65 : h * 65 + 64] = b_qkv[vcols]
            b_v[h * 65 + 64] = 1.0  # ones column (weight col stays 0)
        w_v = np.ascontiguousarray(
            w_v.reshape(KO, 128, DVE_).swapaxes(0, 1).astype(ml_dtypes.bfloat16)
        )
        b_v_bc = np.ascontiguousarray(np.tile(b_v[None, :], (128, 1)))

        w_o_g = w_o[g * 256 : (g + 1) * 256, :]  # [256, D]
        w_o_g = np.ascontiguousarray(
            w_o_g.reshape(2, 128, D).swapaxes(0, 1).astype(ml_dtypes.bfloat16)
        )

        in_maps.append(
            {
                "xT": xT[b],
                "w_qk": w_qk,
                "b_qk": b_qk,
                "w_v": w_v,
                "b_v": b_v_bc,
                "w_o": w_o_g,
            }
        )

    trace = bool(os.environ.get("KERNEL_TRACE"))
    res = run_bass_kernel_spmd(nc, in_maps, core_ids=list(range(8)), trace=trace)
    LAST_RESULT = res

    out = np.empty((B, T, D), dtype=np.float32)
    for b in range(B):
        acc = res.results[b * 4]["out_part"].astype(np.float32).copy()
        for g in range(1, 4):
            acc += res.results[b * 4 + g]["out_part"]
        out[b] = acc + b_o[None, :]
    return out
